# revision 7
# baseline (speedup 1.0000x reference)
"""Multi-head attention with 2D RoPE on 8 Trainium2 NeuronCores.

Problem (hardcoded): B=8, L=1024, EMB=768, 12 heads x 64 dim, 2D RoPE
(x/y tables of length 32, base 100), softmax attention, output projection.

Sharding: data-parallel over batch - one batch element per core, no
collectives.

v4 design (evolved from the v3 PE-roofline kernel; 184.2us -> 167.6us):

  * q/k/v projections run as scaled hi/lo fp8e4m3 DoubleRow matmuls:
    x ~ hi + lo with both parts fp8; W*64 and emb*8 pre-scaling keeps
    the lo parts out of fp8-subnormal territory, and the 3 cross terms
    (Wh@Eh + Wh@El + Wl@Eh) give ~2x better accuracy than bf16 at 0.75x
    the PE cycles (DoubleRow contracts 2x128 rows at 0.5 cyc/row; the
    hi/lo pair is packed in one tile so each weight slice is one DMA).
    The net (64*8)^2 scale is folded into the exp() scale constant and
    into Wp; attention scores themselves stay bf16 (plain fp8 q/k/exp/v
    all fail the 2e-2 budget - measured, not guessed).
  * rope: one fused DVE multiply produces q*ssh and q*cos together
    (stride-0 repeat of the PSUM operand against a [128, 2, L] bf16
    ssh/cos table), the PE applies the 16-lane swap R as a matmul in
    place, one DVE add writes the bf16 qT/kT. The DVE multiply is
    emitted a j-group ahead of the PE rotation.
  * steady loop per (pair, half): 8 j-groups of [scores -> carried AVs
    -> next-pair projection part -> lag-4 AV]; the last 4 AVs and the
    normalize of each half are carried into the next half so only
    exp-dependent work ever sits at the tail of the in-order PE queue.
    Transposes of the previous half's normalize run at j3.
  * av2 PSUM accumulators are never memset: the first AV matmul of each
    av2 tile uses start=True, whose pending-zero region covers the whole
    bank and later sub-region matmuls write through. Same for the shared
    softmax-denominator bank (96 ap-1 matmul columns, one per
    (pair, half, qtile, head)).
  * output projection is 3-staged (pairs 0-2 partial + bias, pairs 3-4
    updates, pair-5 finish) with the partial/update stages spread over
    the projection-free final pair's halves; outputs stage through bf16
    and are cast back to f32 on the host.
  * all inputs are fp8/bf16 (6.6MB vs 15.5MB f32r), startup DMAs are
    ordered so the hi*hi projection term starts as soon as possible, and
    a PE warmup chain keeps the clock-gate ramped through the DMA head.

Engine budget (TimelineSim): PE ~123us busy (bottleneck), ACT exp
96x1038ns ~101us, DVE ~81us, DMA ~25us, Pool idle. Schedule knobs in
DEFAULT_SCHED were tuned by greedy search over TimelineSim; correctness
rel err 7.0e-3 vs the 2e-2 budget.
"""

import numpy as np

import concourse.bass as bass
import concourse.mybir as mybir
import concourse.tile as tile
from concourse import bacc
from concourse.bass import ts
from concourse.bass_utils import run_bass_kernel_spmd

F32 = mybir.dt.float32
BF16 = mybir.dt.bfloat16
FP8 = mybir.dt.float8e4
AF = mybir.ActivationFunctionType
DR = mybir.MatmulPerfMode.DoubleRow

HEAD_NUM = 12
EMB = 768
HEAD = 64
L = 1024
B = 8
X_SIZE = 32
Y_SIZE = 32
BASE = 100.0
N_CORES = 8

KT = EMB // 128   # 6 contraction tiles over channels
NJ = L // 128     # 8 key tiles
NPAIR = HEAD_NUM // 2  # 6 head pairs

A_W = 64.0        # weight pre-scale for fp8 hi/lo range
A_E = 8.0         # emb pre-scale
ESCALE = (HEAD ** -0.5) / (A_W * A_E) ** 2  # folded into exp()

# hi/lo fp8 DR term order: (Wh,Eh), (Wh,El), (Wl,Eh)
TERMS = ((0, 0), (0, 1), (1, 0))


def _tables_np(pos_len, d, base=BASE):
    inv_freq = 1.0 / base ** (np.arange(0, d, 2, dtype=np.float32) / d)
    freqs = np.outer(np.arange(pos_len, dtype=np.float32), inv_freq)
    freqs = np.concatenate([freqs, freqs], axis=-1)
    return np.sin(freqs).astype(np.float32), np.cos(freqs).astype(np.float32)


def _rope_coeffs(pos):
    """cos128/ssh128: [128, L] elementwise RoPE coefficients, 2 heads deep.

    Row layout per 64-row head block: rows 0:32 x-part, rows 32:64 y-part.
    ssh is the sin table pre-shifted/negated so that
        rope(q) = q * cos128 + R128 @ (q * ssh128)
    where R128 swaps 16-row halves within each 32-row block.
    """
    sx, cx = _tables_np(X_SIZE, HEAD // 2)
    sy, cy = _tables_np(Y_SIZE, HEAD // 2)
    px, py = pos[:, 0], pos[:, 1]
    cosxT = cx[px].T  # [32, L]
    cosyT = cy[py].T
    sinxT = sx[px].T
    sinyT = sy[py].T

    def shift(s):
        out = np.empty_like(s)
        out[0:16] = s[16:32]
        out[16:32] = -s[0:16]
        return out

    cos64 = np.concatenate([cosxT, cosyT], axis=0)          # [64, L]
    ssh64 = np.concatenate([shift(sinxT), shift(sinyT)], axis=0)
    cos128 = np.concatenate([cos64, cos64], axis=0).astype(np.float32)
    ssh128 = np.concatenate([ssh64, ssh64], axis=0).astype(np.float32)
    return np.ascontiguousarray(cos128), np.ascontiguousarray(ssh128)


def _r128():
    r32 = np.zeros((32, 32), dtype=np.float32)
    for d in range(16):
        r32[d, d + 16] = 1.0
        r32[d + 16, d] = 1.0
    return np.kron(np.eye(4, dtype=np.float32), r32)


DEFAULT_SCHED = dict(av_lag=4, carry_first=False, proj_t=3, tp=(3,),
                     pair0="ser", fp="p5", store="dve", norm_lag=1,
                     tp_next=False)


def build_nc(sched=None):
    sched = dict(DEFAULT_SCHED, **(sched or {}))
    nc = bacc.Bacc()
    embp = nc.declare_dram_parameter("embp", [128, 2, 2, KT, 512],
                                     FP8, isOutput=False)
    wqp = nc.declare_dram_parameter("wqp", [NPAIR, 128, 2, KT, 128], FP8,
                                    isOutput=False)
    wkp = nc.declare_dram_parameter("wkp", [NPAIR, 128, 2, KT, 128], FP8,
                                    isOutput=False)
    wvp = [nc.declare_dram_parameter(f"wv{s}", [128, KT, EMB], FP8,
                                     isOutput=False) for s in "hl"]
    wp = nc.declare_dram_parameter("wp", [EMB, EMB], BF16, isOutput=False)
    bp = nc.declare_dram_parameter("bp", [1, EMB], BF16, isOutput=False)
    sc = nc.declare_dram_parameter("sc", [128, 2, 2, 512], BF16,
                                    isOutput=False)
    # packed constants: r128 + ident
    consts = nc.declare_dram_parameter("consts", [128, 256], BF16,
                                       isOutput=False)
    out = nc.declare_dram_parameter("out", [L, EMB], BF16, isOutput=True)

    with tile.TileContext(nc) as tc:
        with (
            tc.tile_pool(name="const", bufs=1) as p_const,
            tc.tile_pool(name="vaug", bufs=1) as p_vaug,
            tc.tile_pool(name="persist", bufs=1) as p_per,
            tc.tile_pool(name="wsl", bufs=2) as p_wsl,
            tc.tile_pool(name="qk", bufs=2) as p_qk,
            tc.tile_pool(name="tsc", bufs=2) as p_tsc,
            tc.tile_pool(name="exp", bufs=12) as p_exp,
            tc.tile_pool(name="avsb", bufs=3) as p_avsb,
            tc.tile_pool(name="rsb", bufs=3) as p_rsb,
            tc.tile_pool(name="outp", bufs=6) as p_out,
            tc.tile_pool(name="opart", bufs=8) as p_opart,
            tc.tile_pool(name="big", bufs=2, space="PSUM") as ps_big,
            tc.tile_pool(name="qp", bufs=1, space="PSUM") as ps_qp,
            tc.tile_pool(name="av", bufs=2, space="PSUM") as ps_av,
            tc.tile_pool(name="sum", bufs=1, space="PSUM") as ps_sum,
        ):
            avT_t = [p_per.tile([128, NJ, 128], BF16, tag=f"avT{p}",
                                name=f"avT{p}") for p in range(NPAIR)]

            # per-pair q/k weight slices, hi+lo packed in one fp8 tile
            def load_wslice(w_dram, pair, wtag):
                wsl = p_wsl.tile([128, 2, KT, 128], FP8, tag=wtag,
                                 name=f"wsl{wtag}{pair}")
                eng = nc.sync if wtag == "q" else nc.scalar
                eng.dma_start(wsl[:], w_dram[pair])
                return wsl

            # ---- startup loads; hi parts first so the hi*hi projection
            # term can start as soon as possible, embT in column halves.
            PRE_Q0 = load_wslice(wqp, 0, "q")

            # PE warmup during the DMA head keeps the HAM clock-gate warm
            # (DVE memset so the warmup isn't gated on a slow Q7 launch)
            wu = p_const.tile([128, 64], F32, tag="warm")
            nc.vector.memset(wu[:], 0.0)
            wup = ps_qp.tile([128, 512], F32, tag="qp", name="warmps")
            for _ in range(14):
                nc.tensor.matmul(wup[0:64, 0:64], wu[:, 0:64], wu[:],
                                 start=True, stop=True)

            embt = p_per.tile([128, 2, 2, KT, 512], FP8, tag="emb",
                              name="embt")
            nc.sync.dma_start(embt[:, 0, 0], embp[:, 0, 0])
            cpack = p_const.tile([128, 256], BF16, tag="cpack")
            r_t = cpack[:, 0:128]
            id_t = cpack[:, 128:256]
            nc.scalar.dma_start(cpack[:], consts[:])
            sc_t = p_const.tile([128, 2, 2, 512], BF16, tag="sc")
            nc.scalar.dma_start(sc_t[:, 0], sc[:, 0])
            nc.sync.dma_start(embt[:, 1, 0], embp[:, 1, 0])
            PRE_K0 = load_wslice(wkp, 0, "k")
            nc.scalar.dma_start(sc_t[:, 1], sc[:, 1])
            for s in range(2):
                eng = nc.sync if s == 0 else nc.scalar
                eng.dma_start(embt[:, s, 1], embp[:, s, 1])
            PRE_Q1 = load_wslice(wqp, 1, "q")
            PRE_K1 = load_wslice(wkp, 1, "k")
            wv_t = []
            for s in range(2):
                w = p_per.tile([128, KT, EMB], FP8, tag=f"wv{s}",
                               name=f"wvt{s}")
                eng = nc.sync if s == 0 else nc.scalar
                eng.dma_start(w[:], wvp[s][:])
                wv_t.append(w)
            wp_t = [p_per.tile([128, EMB], BF16, tag=f"wpp{k}",
                               name=f"wpt{k}") for k in range(KT)]
            for k in range(KT):
                nc.sync.dma_start(wp_t[k][:], wp[ts(k, 128), :])
            bpb_t = p_const.tile([128, EMB], BF16, tag="bpb")
            nc.sync.dma_start(bpb_t[:], bp[:].to_broadcast((128, EMB)))

            # ones moving-vector for the ap-1 softmax-denominator matmuls
            ones_mv = p_const.tile([128, 1], BF16, tag="ones")
            nc.gpsimd.memset(ones_mv[:], 1.0)

            # one shared PSUM bank of [128, 1] denominator accumulators,
            # region col = pair*16 + half*8 + qs*2 + head. Zeroed by the
            # very first sums matmul's start=True pending-zero region.
            sums_ps = ps_sum.tile([128, 96], F32, tag="sums")
            sums_started = [False]

            vaug_t = [p_vaug.tile([128, EMB], BF16, tag=f"vaug{j}",
                                  name=f"vaug{j}")
                      for j in range(NJ)]

            # ---- hi/lo DR projection: 9 matmuls per [128, 512] chunk
            def proj_mm(qp, wsl, c0, lo=0, hi=9):
                i = 0
                for (sw, se) in TERMS:
                    for t in range(KT // 2):
                        if lo <= i < hi:
                            nc.tensor.matmul(
                                qp,
                                wsl[:, sw, 2 * t:2 * t + 2, :],
                                embt[:, se, c0 // 512, 2 * t:2 * t + 2],
                                start=(i == 0), stop=(i == 8),
                                perf_mode=DR,
                            )
                        i += 1

            def rope_into(dst, qp, c0):
                """dst[:, c0:c0+512] = rope(proj chunk in qp); the rotation
                matmul overwrites the qp PSUM in place."""
                t_sc = p_tsc.tile([128, 2, 512], BF16, tag="tsc")
                src = qp[:][:, None, :].to_broadcast((128, 2, 512))
                nc.vector.tensor_mul(t_sc[:], src, sc_t[:, c0 // 512])
                nc.tensor.matmul(qp[:], r_t, t_sc[:, 0, :],
                                 start=True, stop=True)
                nc.vector.tensor_add(dst[:, c0:c0 + 512], t_sc[:, 1, :],
                                     qp[:])

            # rope'd projection, emission split into parts so the PE work
            # spreads evenly over the j-loop; the DVE rope multiply is
            # emitted a group ahead of the PE rotation so the in-order PE
            # queue never waits on it
            def make_proj(pair, wtag, wsl):
                dst = p_qk.tile([128, L], BF16, tag=wtag,
                                name=f"{wtag}T{pair}")
                qps = {}
                tscs = {}

                def mm(ci, part):
                    if part == 0:
                        qps[ci] = ps_qp.tile([128, 512], F32, tag="qp",
                                             name=f"qp{wtag}{pair}{ci}")
                    lo, hi = (0, 5) if part == 0 else (5, 9)
                    proj_mm(qps[ci][:], wsl, ci * 512, lo, hi)

                def rope_mul(ci):
                    c0 = ci * 512
                    t_sc = p_tsc.tile([128, 2, 512], BF16, tag="tsc")
                    src = qps[ci][:][:, None, :].to_broadcast((128, 2, 512))
                    nc.vector.tensor_mul(t_sc[:], src,
                                         sc_t[:, c0 // 512])
                    tscs[ci] = t_sc

                def rope_fin(ci):
                    c0 = ci * 512
                    t_sc = tscs[ci]
                    nc.tensor.matmul(qps[ci][:], r_t, t_sc[:, 0, :],
                                     start=True, stop=True)
                    nc.vector.tensor_add(dst[:, c0:c0 + 512],
                                         t_sc[:, 1, :], qps[ci][:])
                return dst, mm, rope_mul, rope_fin

            # v projection for one key tile -> vaug[j] (bf16); channel
            # layout already matches the flipped-AV moving operand
            def project_v(j):
                vp = ps_big.tile([128, L], F32, tag="big", name=f"vp{j}")
                for c0, c1 in ((0, 512), (512, 768)):
                    i = 0
                    for (sw, se) in TERMS:
                        for t in range(KT // 2):
                            nc.tensor.matmul(
                                vp[:, c0:c1],
                                embt[:, se, j // 4, 2 * t:2 * t + 2,
                                     ts(j % 4, 128)],
                                wv_t[sw][:, 2 * t:2 * t + 2, c0:c1],
                                start=(i == 0), stop=(i == 8),
                                perf_mode=DR,
                            )
                            i += 1
                nc.vector.tensor_copy(vaug_t[j][:], vp[:, 0:EMB])

            imm = sched["store"] == "imm"

            # PSUM -> bf16 staging -> store; in "imm" mode the bias/partial
            # ride the PE and the copy alternates DVE/ACT
            def store_out(qt, fp, add=None):
                o_sb = p_out.tile([128, EMB], BF16, tag="osb",
                                  name=f"osb{qt}")
                if add is not None:
                    nc.vector.tensor_add(o_sb[:], fp[:, 0:EMB], add)
                elif qt % 2 == 0:
                    nc.vector.tensor_copy(o_sb[:], fp[:, 0:EMB])
                else:
                    nc.scalar.copy(o_sb[:], fp[:, 0:EMB])
                oeng = nc.sync if qt % 2 == 0 else nc.scalar
                oeng.dma_start(out[ts(qt, 128), :], o_sb[:])

            # final projection for one 128-query tile (+bias, store)
            def fp_qtile(qt):
                fp = ps_big.tile([128, L], F32, tag="big", name=f"fp{qt}")
                for c0, c1 in ((0, 512), (512, 768)):
                    for pp in range(NPAIR):
                        nc.tensor.matmul(
                            fp[:, c0:c1],
                            avT_t[pp][:, qt],
                            wp_t[pp][:, c0:c1],
                            start=(pp == 0),
                            stop=(False if imm else pp == NPAIR - 1),
                        )
                    if imm:
                        nc.tensor.matmul(fp[:, c0:c1], id_t,
                                         bpb_t[:, c0:c1],
                                         start=False, stop=True)
                store_out(qt, fp, None if imm else bpb_t[:])

            # split final projection: pairs 0-3 (+bias) pre-accumulated
            # early into SBUF, later pairs folded in, pair 5 at the tail
            oparts = {}

            def fp_partial(qt):
                fp = ps_big.tile([128, L], F32, tag="big", name=f"fpp{qt}")
                for c0, c1 in ((0, 512), (512, 768)):
                    for pp in range(3):
                        nc.tensor.matmul(
                            fp[:, c0:c1],
                            avT_t[pp][:, qt],
                            wp_t[pp][:, c0:c1],
                            start=(pp == 0), stop=(False if imm else pp == 2),
                        )
                    if imm:
                        nc.tensor.matmul(fp[:, c0:c1], id_t,
                                         bpb_t[:, c0:c1],
                                         start=False, stop=True)
                o_part = p_opart.tile([128, EMB], BF16, tag="opart",
                                      name=f"opart{qt}")
                if imm:
                    nc.vector.tensor_copy(o_part[:], fp[:, 0:EMB])
                else:
                    nc.vector.tensor_add(o_part[:], fp[:, 0:EMB], bpb_t[:])
                oparts[qt] = o_part

            def fp_update(qt, plo, phi):
                fp = ps_big.tile([128, L], F32, tag="big",
                                 name=f"fpu{qt}_{plo}")
                for c0, c1 in ((0, 512), (512, 768)):
                    for pp in range(plo, phi + 1):
                        nc.tensor.matmul(
                            fp[:, c0:c1],
                            avT_t[pp][:, qt],
                            wp_t[pp][:, c0:c1],
                            start=(pp == plo), stop=(pp == phi),
                        )
                nc.vector.tensor_add(oparts[qt][:], oparts[qt][:],
                                     fp[:, 0:EMB])

            def fp_finish(qt):
                fp = ps_big.tile([128, L], F32, tag="big", name=f"fpf{qt}")
                for c0, c1 in ((0, 512), (512, 768)):
                    nc.tensor.matmul(
                        fp[:, c0:c1],
                        avT_t[NPAIR - 1][:, qt],
                        wp_t[NPAIR - 1][:, c0:c1],
                        start=True, stop=(not imm),
                    )
                    if imm:
                        nc.tensor.matmul(fp[:, c0:c1], id_t,
                                         oparts[qt][:, c0:c1],
                                         start=False, stop=True)
                store_out(qt, fp, None if imm else oparts[qt][:])

            # ---- attention building blocks -------------------------------

            def emit_scores_exp(pair, half, j, qT, kT):
                """scores + exp for key tile j over this half's 512 queries
                per head. Head h0 lands in sAB bank 0, h1 in bank 1."""
                qlo = 512 * half
                sAB = ps_big.tile([128, L], F32, tag="big",
                                  name=f"s{pair}_{half}_{j}")
                for hh in range(2):
                    p0 = 64 * hh
                    nc.tensor.matmul(
                        sAB[:, 512 * hh:512 * hh + 512],
                        kT[p0:p0 + 64, ts(j, 128)],
                        qT[p0:p0 + 64, qlo:qlo + 512],
                        start=True, stop=True,
                        tile_position=(p0, 0),
                    )
                expt = p_exp.tile([128, L], BF16, tag="expt",
                                  name=f"e{pair}_{half}_{j}")
                nc.scalar.activation(expt[:], sAB[:], AF.Exp, scale=ESCALE)
                return expt

            av2_started = {}

            def emit_av(pair, half, av2, j, expt):
                """flipped AV + denominator matmuls; expt holds 2 heads x
                512 queries packed [h0 | h1]."""
                key = id(av2)
                for qs in range(4):
                    for hh in range(2):
                        e_sl = expt[:, 512 * hh + 128 * qs:
                                    512 * hh + 128 * qs + 128]
                        first = not av2_started.get(key, False)
                        av2_started[key] = True
                        nc.tensor.matmul(
                            av2[:, qs * 128 + hh * 64:
                                qs * 128 + hh * 64 + 64],
                            e_sl,
                            vaug_t[j][:, (2 * pair + hh) * 64:
                                      (2 * pair + hh) * 64 + 64],
                            start=first, stop=(j == NJ - 1),
                            skip_group_check=True,
                        )
                        sc_ = pair * 16 + half * 8 + qs * 2 + hh
                        sfirst = not sums_started[0]
                        sums_started[0] = True
                        nc.tensor.matmul(
                            sums_ps[:, sc_:sc_ + 1],
                            e_sl,
                            ones_mv[:],
                            start=sfirst, stop=(j == NJ - 1),
                            skip_group_check=True,
                        )

            def emit_norm(pair, half, av2):
                """reciprocal + scale-copy av2 -> avsb bf16."""
                soff = pair * 16 + half * 8
                r_sb = p_rsb.tile([128, 8], F32, tag="rsb",
                                  name=f"r{pair}_{half}")
                nc.vector.reciprocal_approx_fast(
                    r_sb[:, 0:8], sums_ps[:, soff:soff + 8])
                avsb = p_avsb.tile([128, 4, 128], BF16, tag="avsb",
                                   name=f"avsb{pair}_{half}")
                rb = r_sb[:, 0:8][:, :, None].to_broadcast((128, 8, 64))
                nc.vector.tensor_mul(
                    avsb[:].rearrange("p a b -> p (a b)")
                    .rearrange("p (a b) -> p a b", b=64),
                    av2[:].rearrange("p (a b) -> p a b", b=64),
                    rb)
                return avsb

            def make_transposes(pair, half, avsb, on_act):
                def run(qlo=0, qhi=4):
                    tp = ps_big.tile([128, L], F32, tag="big",
                                     name=f"tp{pair}_{half}_{qlo}")
                    for qs in range(qlo, qhi):
                        sub = tp[:, 128 * qs:128 * qs + 64].bitcast(BF16)
                        nc.tensor.transpose(sub, avsb[:, qs, :], id_t)
                        dst = avT_t[pair][:, 4 * half + qs, :]
                        if on_act:
                            nc.scalar.copy(dst, sub)
                        else:
                            nc.vector.tensor_copy(dst, sub)
                return run

            # ---- pair 0 startup: chunk-ordered projections + special
            # half-0 (scores first, v-projections once wv arrives). The
            # qp pool has a single slot; the k/q second chunks borrow the
            # av pool, which is idle until the v loop.
            qT = p_qk.tile([128, L], BF16, tag="q", name="qT0")
            kT = p_qk.tile([128, L], BF16, tag="k", name="kT0")
            qp_q0 = ps_qp.tile([128, 512], F32, tag="qp", name="qp_q0c0")
            proj_mm(qp_q0[:], PRE_Q0, 0, 0, 3)
            kp0 = ps_av.tile([128, 512], F32, tag="av2", name="kp0")
            proj_mm(kp0[:], PRE_K0, 0, 0, 3)
            proj_mm(qp_q0[:], PRE_Q0, 0, 3, 9)
            proj_mm(kp0[:], PRE_K0, 0, 3, 9)

            rope_into(qT, qp_q0, 0)
            rope_into(kT, kp0, 0)

            # half-0 scores for key tiles 0..3 (kT chunk 0 only)
            p0h0_exps = []
            for j in range(4):
                p0h0_exps.append(emit_scores_exp(0, 0, j, qT, kT))
            # chunk-1 projections (paced by the embT second-half DMAs)
            qp_q1 = ps_av.tile([128, 512], F32, tag="av2", name="qp_q0c1")
            proj_mm(qp_q1[:], PRE_Q0, 512)
            kp1 = ps_qp.tile([128, 512], F32, tag="qp", name="kp1")
            proj_mm(kp1[:], PRE_K0, 512)
            rope_into(qT, qp_q1, 512)
            rope_into(kT, kp1, 512)
            for j in range(4, NJ):
                p0h0_exps.append(emit_scores_exp(0, 0, j, qT, kT))

            av2 = ps_av.tile([128, 512], F32, tag="av2", name="av2_0_0")
            # pair-1 q projection fills the gap until wv arrives
            qT_n, qn_mm, qn_rmul, qn_rfin = make_proj(1, "q", PRE_Q1)
            kT_n, kn_mm, kn_rmul, kn_rfin = make_proj(1, "k", PRE_K1)
            qn_mm(0, 0)
            qn_mm(0, 1)
            qn_rmul(0)
            qn_rfin(0)
            qn_mm(1, 0)
            qn_mm(1, 1)
            qn_rmul(1)
            qn_rfin(1)

            lag = sched["av_lag"]
            nlag = sched["norm_lag"]
            # v projections with the (0,1) scores riding along, so ACT
            # chews through the next 8 exps while the PE does the v
            # matmuls; half-0 AV deferred one v tile so the DVE bf16 copy
            # has drained
            if sched["pair0"] == "ilv":
                exps01 = []
                for j in range(NJ):
                    project_v(j)
                    exps01.append(emit_scores_exp(0, 1, j, qT, kT))
                    if j >= 1:
                        emit_av(0, 0, av2, j - 1, p0h0_exps[j - 1])
                emit_av(0, 0, av2, NJ - 1, p0h0_exps[NJ - 1])
                p0h0_exps = None
                avsb = emit_norm(0, 0, av2)
                av2b = ps_av.tile([128, 512], F32, tag="av2",
                                  name="av2_0_1")
                tp00 = make_transposes(0, 0, avsb, False)
                # (0,1) AV drain interleaved with the pair-1 k projection
                kn_mm(0, 0)
                emit_av(0, 1, av2b, 0, exps01[0])
                emit_av(0, 1, av2b, 1, exps01[1])
                kn_mm(0, 1)
                emit_av(0, 1, av2b, 2, exps01[2])
                kn_rmul(0)
                kn_rfin(0)
                tp00(0, 4)
                kn_mm(1, 0)
                emit_av(0, 1, av2b, 3, exps01[3])
                emit_av(0, 1, av2b, 4, exps01[4])
                kn_mm(1, 1)
                kn_rmul(1)
                emit_av(0, 1, av2b, 5, exps01[5])
                kn_rfin(1)
                carry = [(0, 1, av2b, j, exps01[j]) for j in (6, 7)]
                norm_q = [(0, 1, av2b)]
                exps01 = None
                av2 = ps_av.tile([128, 512], F32, tag="av2",
                                 name="av2_1_0")
                qT, kT = qT_n, kT_n
                tps = []
                halves = [(p, h) for p in range(1, NPAIR) for h in (0, 1)]
            else:
                for j in range(NJ):
                    project_v(j)
                    if j >= 1:
                        emit_av(0, 0, av2, j - 1, p0h0_exps[j - 1])
                emit_av(0, 0, av2, NJ - 1, p0h0_exps[NJ - 1])
                p0h0_exps = None
                av2_next = ps_av.tile([128, 512], F32, tag="av2",
                                      name="av2_0_1")
                # each half's normalize is deferred `norm_lag` halves so
                # its carried AVs (and the ACT exp backlog they wait on)
                # are long done by the time the norm chain runs
                norm_q = [(0, 0, av2)]
                tps = []      # pending (runner, {j: (lo, hi)}) transposes
                av2 = av2_next
                carry = []
                halves = [(p, h) for p in range(NPAIR) for h in (0, 1)][1:]
            for pair, half in halves:
                last = (pair == NPAIR - 1 and half == 1)
                if half == 0 and pair + 1 < NPAIR:
                    wsl_q = load_wslice(wqp, pair + 1, "q")
                    wsl_k = load_wslice(wkp, pair + 1, "k")
                    qT_n, qn_mm, qn_rmul, qn_rfin = \
                        make_proj(pair + 1, "q", wsl_q)
                    kT_n, kn_mm, kn_rmul, kn_rfin = \
                        make_proj(pair + 1, "k", wsl_k)

                # projection part placement templates: entries are lists of
                # (kind, arg) with kinds mmA/mmB (chunk parts), rm/rf (rope)
                PROJ_T = {
                    0: {1: [("A", 0)], 2: [("B", 0)], 3: [("m", 0), ("f", 0)],
                        4: [("A", 1)], 5: [("B", 1)],
                        6: [("m", 1), ("f", 1)]},
                    1: {1: [("A", 0)], 2: [("B", 0), ("m", 0)],
                        3: [("f", 0), ("A", 1)], 4: [("B", 1), ("m", 1)],
                        5: [("f", 1)]},
                    2: {1: [("A", 0)], 2: [("B", 0)], 3: [("m", 0), ("A", 1)],
                        4: [("f", 0), ("B", 1)], 5: [("m", 1)],
                        6: [("f", 1)]},
                    3: {1: [("A", 0), ("B", 0)], 2: [("m", 0)],
                        3: [("f", 0)], 4: [("A", 1), ("B", 1)],
                        5: [("m", 1)], 6: [("f", 1)]},
                }[sched["proj_t"]]
                tp_sched = sched["tp"]

                def do_norm(tp_place, defer=0):
                    p_, h_, a_ = norm_q.pop(0)
                    avsb = emit_norm(p_, h_, a_)
                    tps.append([make_transposes(p_, h_, avsb, False),
                                dict(tp_place), defer])

                pend = []
                for j in range(NJ):
                    def drain_carry(n):
                        for _ in range(min(n, len(carry))):
                            emit_av(*carry.pop(0))

                    if sched["carry_first"]:
                        if j == 0:
                            drain_carry(1)
                        if j == 1:
                            drain_carry(9)
                        expt = emit_scores_exp(pair, half, j, qT, kT)
                    else:
                        expt = emit_scores_exp(pair, half, j, qT, kT)
                        if j == 0:
                            drain_carry(1)
                        if j == 1:
                            drain_carry(9)
                    if j == 1 and len(norm_q) >= nlag:
                        if sched["tp_next"] and not (
                                pair == NPAIR - 1):
                            # transposes run next half, where avsb is
                            # unconditionally ready
                            do_norm({0: (0, 2), 1: (2, 4)}, defer=1)
                        elif len(tp_sched) == 1:
                            do_norm({tp_sched[0]: (0, 4)})
                        else:
                            do_norm({tp_sched[0]: (0, 2),
                                     tp_sched[1]: (2, 4)})
                    if j == 5 and last and norm_q:
                        # the (5,0) normalize squeezed before the epilogue
                        do_norm({6: (0, 2), 7: (2, 4)})
                    # next-pair projection chunks (q during half 0,
                    # k during half 1), spread across the loop
                    if not last and pair + 1 < NPAIR:
                        mmf, rmulf, rfinf = (
                            (qn_mm, qn_rmul, qn_rfin) if half == 0
                            else (kn_mm, kn_rmul, kn_rfin))
                        for kind, ci in PROJ_T.get(j, []):
                            if kind == "A":
                                mmf(ci, 0)
                            elif kind == "B":
                                mmf(ci, 1)
                            elif kind == "m":
                                rmulf(ci)
                            else:
                                rfinf(ci)
                    # lagged AV sits at the tail of the j group so only
                    # exp-dependent work can ever wait
                    if len(pend) >= sched["av_lag"]:
                        emit_av(pair, half, av2, *pend.pop(0))
                    pend.append((j, expt))
                    for item in tps:
                        if item[2] == 0:
                            seg = item[1].pop(j, None)
                            if seg is not None:
                                item[0](*seg)
                    tps = [it for it in tps if it[1]]
                    # half-1 output-projection partial placement; all avT
                    # reads here are >= norm_lag halves behind the norms
                    if nlag == 1 and sched["fp"] == "pipe":
                        # fully pipelined: partials over (3,1)..(5,0),
                        # pair-3/4 updates in the last two halves, qt0-3
                        # finished (and stored) inside the last loop
                        fppq = {(3, 1): {4: 4, 6: 5}, (4, 0): {2: 6, 6: 7},
                                (4, 1): {2: 0, 6: 1}, (5, 0): {2: 2, 6: 3}}
                        qt = fppq.get((pair, half), {}).get(j)
                        if qt is not None:
                            fp_partial(qt)
                        if pair == NPAIR - 1 and half == 0 and j >= 4:
                            fp_update(j, 3, 4)
                        if last and j < 4:
                            fp_update(j, 3, 4)
                        if last and j >= 4:
                            fp_finish(j - 4)
                    elif nlag == 1 and sched["fp"] == "spread":
                        fpq = {(3, 1): 4, (4, 0): 5, (4, 1): 6, (5, 0): 7}
                        if j == 2 and (pair, half) in fpq:
                            fp_partial(fpq[(pair, half)])
                        if pair == NPAIR - 1 and half == 0 and j < 4:
                            fp_update(4 + j, 3, 3)
                        if last and j < 4:
                            fp_update(4 + j, 4, 4)
                    elif nlag == 1:
                        if pair == NPAIR - 1 and half == 0:
                            if j < 4:
                                fp_partial(4 + j)
                            else:
                                fp_update(j, 3, 4)
                    else:
                        fpq = {(4, 0): {2: 4, 6: 5}, (4, 1): {2: 6, 6: 7}}
                        qt = fpq.get((pair, half), {}).get(j)
                        if qt is not None:
                            fp_partial(qt)
                        if pair == NPAIR - 1 and half == 0 and j < 4:
                            fp_update(4 + j, 3, 3)
                        if last and j >= 4:
                            fp_update(j, 4, 4)
                for item in tps:
                    item[2] = max(0, item[2] - 1)
                if not last:
                    carry = [(pair, half, av2, jj, ee) for jj, ee in pend]
                    norm_q.append((pair, half, av2))
                    av2_next = ps_av.tile([128, 512], F32, tag="av2",
                                          name=f"av2n_{pair}_{half}")
                    av2 = av2_next
                    if half == 1 and pair + 1 < NPAIR:
                        qT, kT = qT_n, kT_n
                else:
                    # epilogue: in pipe mode qt0-3 already stored in-loop
                    if sched["fp"] != "pipe":
                        for qt in range(4):
                            fp_qtile(qt)
                    for item in pend:
                        emit_av(pair, half, av2, *item)
                    avsb = emit_norm(pair, half, av2)
                    make_transposes(pair, half, avsb, True)()
                    for qt in range(4, NJ):
                        fp_finish(qt)

    nc.finalize()
    return nc


_NC_CACHE = {}


def _get_nc(sched=None):
    key = tuple(sorted(dict(DEFAULT_SCHED, **(sched or {})).items()))
    if key not in _NC_CACHE:
        _NC_CACHE[key] = build_nc(sched)
    return _NC_CACHE[key]


def kernel(emb, pos, Wq, Wk, Wv, Wp, bp, _trace=False, _cores=N_CORES):
    import ml_dtypes

    FP8NP = ml_dtypes.float8_e4m3

    emb = np.asarray(emb, dtype=np.float32)
    pos = np.asarray(pos)

    def hilo(x):
        hi = x.astype(FP8NP)
        lo = (x - hi.astype(np.float32)).astype(FP8NP)
        return hi, lo

    def swizzle_qk(w):
        # [EMB, EMB] -> [NPAIR, 128, KT, 128]: pair cols, contraction
        # row-tile t on free dim 1
        return np.ascontiguousarray(
            w.reshape(KT, 128, NPAIR, 128).transpose(2, 1, 0, 3))

    def swizzle_v(w):
        # [EMB, EMB] -> [128, KT, EMB] moving layout
        return np.ascontiguousarray(
            w.reshape(KT, 128, EMB).transpose(1, 0, 2))

    Wqh, Wql = hilo(swizzle_qk(np.asarray(Wq, dtype=np.float32) * A_W))
    Wkh, Wkl = hilo(swizzle_qk(np.asarray(Wk, dtype=np.float32) * A_W))
    Wvh, Wvl = hilo(swizzle_v(np.asarray(Wv, dtype=np.float32) * A_W))
    Wq_p = np.ascontiguousarray(np.stack([Wqh, Wql], axis=2))
    Wk_p = np.ascontiguousarray(np.stack([Wkh, Wkl], axis=2))
    Wp_b = (np.asarray(Wp, dtype=np.float32) / (A_W * A_E)).astype(
        ml_dtypes.bfloat16)
    bp2 = np.asarray(bp, dtype=np.float32).reshape(1, EMB).astype(
        ml_dtypes.bfloat16)

    cos128, ssh128 = _rope_coeffs(np.asarray(pos))
    sc = np.stack([ssh128, cos128], axis=1)  # [128, 2, L]
    sc = np.ascontiguousarray(
        sc.reshape(128, 2, 2, 512).transpose(0, 2, 1, 3)).astype(
        ml_dtypes.bfloat16)
    consts = np.concatenate(
        [_r128(), np.eye(128, dtype=np.float32)],
        axis=1).astype(ml_dtypes.bfloat16)

    nc = _get_nc()
    in_maps = []
    for b in range(_cores):
        eT = np.ascontiguousarray(emb[b].T) * A_E
        eh, el = hilo(eT.reshape(KT, 128, L).transpose(1, 0, 2))
        ep = np.stack([eh, el], axis=1)  # [128, 2, KT, L]
        ep = ep.reshape(128, 2, KT, 2, 512).transpose(0, 1, 3, 2, 4)
        in_maps.append({
            "embp": np.ascontiguousarray(ep),
            "wqp": Wq_p, "wkp": Wk_p,
            "wvh": Wvh, "wvl": Wvl, "wp": Wp_b, "bp": bp2,
            "sc": sc, "consts": consts,
        })
    res = run_bass_kernel_spmd(nc, in_maps, list(range(_cores)), trace=_trace)
    out = np.stack([res.results[b]["out"].astype(np.float32)
                    for b in range(_cores)], axis=0)
    if _trace:
        return out, res
    return out
